# revision 5
# baseline (speedup 1.0000x reference)
"""Trainium2 Bass kernel for nn_AdaptiveBilateralNetPointwise.

Strategy (8 NeuronCores, SPMD, no collectives):
  - core k handles batch b=k//2, row-half q=k%2 (512 rows x 1024 cols).
  - host ships row-pair sums of the full image (dsum, bf16) so every core
    builds the complete 256x256 lowres locally with a single column-pair
    add per tile; no cross-core dependency, each NEFF runs independently.
  - small conv tower for the batch on TensorE (bf16 matmuls, im2col via
    DMA from zero-padded DRAM staging); g-conv output channels are
    host-permuted to ci-major so each slice coefficient consumes exactly
    one x-interpolated grid tile.
  - bilateral grid (96 ch @ 16x16) expanded to full-x resolution via PE
    matmuls against a host-built interpolation matrix, staged via DRAM
    and reloaded per (block, ci); per 128-row block the y-interp is fused
    into PE matmuls (device-built masked y-weights as stationary),
    drained from PSUM in z-pairs on ScalarE.
  - exact trilinear slice via dense hat-weight contraction over the 8
    luma bins.  Engine placement tuned against HW microbenchmarks:
      * all DVE tensor ops are OUT-OF-PLACE (in-place writes lose the
        dual-port 2x mode: 1.06 ns/elem vs 0.54 measured),
      * hat weights: Act engine Abs (one per z) + two [128,8,1024]-wide
        DVE tensor_scalar ops that run in the 4x mode (0.28 ns/elem),
      * PSUM drains on ScalarE (Act), z-pair granularity, contiguous,
      * hat multiply (z-halves) + half-combine on DVE (2x), second
        reduce level + per-coefficient finish on Pool (gpsimd),
      * apply reads the f32 image directly (2x_2p mode does not require
        16-bit operands, only all-SBUF).
"""
import os
import sys
import numpy as np

sys.path.insert(0, "/opt/trn_rl_repo")

from concourse import bass, bacc, tile, mybir  # noqa: E402
from concourse.bass_utils import run_bass_kernel_spmd  # noqa: E402

F32 = mybir.dt.float32
BF16 = mybir.dt.bfloat16
AF = mybir.ActivationFunctionType
OP = mybir.AluOpType

B, NIN, H, W = 4, 3, 1024, 1024
GB, LB = 16, 8
N_CORES = 8
HALF = 512  # rows per core


def interp_matrix(n_out, n_grid):
    """[n_grid, n_out] bilinear-resize matrix with edge clamping."""
    M = np.zeros((n_grid, n_out), np.float32)
    for i in range(n_out):
        c = (i + 0.5) * (n_grid / n_out) - 0.5
        f = int(np.floor(c))
        t = c - f
        i0 = min(max(f, 0), n_grid - 1)
        i1 = min(max(f + 1, 0), n_grid - 1)
        M[i0, i] += 1.0 - t
        M[i1, i] += t
    return M


def _build_nc(consts):
    """Build the Bass program. consts: dict of host numpy arrays to inline."""
    nc = bacc.Bacc("TRN2", target_bir_lowering=False, debug=False,
                   num_devices=N_CORES)

    # ---------------- external I/O (per-core values) ----------------------
    img = nc.dram_tensor("img", [3, HALF, W], F32, kind="ExternalInput")
    # dsum: full-image bilinear row-pair sums (rows 4l+1 + 4l+2), so every
    # core builds the full 256x256 lowres locally — no collective.
    dsum = nc.dram_tensor("dsum", [3, 256, W], BF16, kind="ExternalInput")
    # dense y-interp weights for this core's row half; masked variant is
    # built on-device by 8 small DMAs.
    wy16 = nc.dram_tensor("wy16", [16, HALF], BF16, kind="ExternalInput")
    val_in = nc.dram_tensor("val", [1, 1], F32, kind="ExternalInput")
    out = nc.dram_tensor("out", [3, HALF, W], F32, kind="ExternalOutput")
    dbg = {}
    _dk = os.environ.get("KDEBUG_KEYS", "")
    if os.environ.get("KDEBUG", "0") == "1":
        for key, shape, dt in (
                ('lr', [6, 128, 256], BF16), ('coeff', [96, 256], BF16),
                ('cz', [128, W], F32), ('gx', [128, W], BF16),
                ('u', [128, 8 * W], BF16), ('tst', [128, 4 * W], BF16),
                ('aff', [128, W], BF16), ('x4', [64, 256], BF16),
                ('splat', [64, 256], BF16)):
            if key in _dk.split(','):
                dbg[key] = nc.dram_tensor(f"d_{key}", shape, dt,
                                          kind="ExternalOutput")

    # ---------------- inlined constants (same on all cores) ---------------
    import ml_dtypes
    const_h = {k: nc.inline_tensor(v.astype(np.float32), name=f"c_{k}")
               for k, v in consts["tensors"].items()}
    const_h["xib"] = nc.inline_tensor(
        consts["tensors"]["xi"].astype(ml_dtypes.bfloat16), name="c_xib")
    imm = consts["imm"]

    # ---------------- internal DRAM staging --------------------------------
    lowpad = nc.dram_tensor("lowpad", [3, 258, 258], BF16)
    a1pad = nc.dram_tensor("a1pad", [8, 130, 130], BF16)
    coeffd = nc.dram_tensor("coeffd", [96, 256], BF16)
    gxd = nc.dram_tensor("gxd", [12, 128, W], BF16)  # x-interp'd grid

    with tile.TileContext(nc) as tc:
        _trace(tc, nc, img, dsum, wy16, val_in, out, const_h, imm,
               (lowpad, a1pad), coeffd, gxd, dbg)
    nc.compile()
    return nc


def _trace(tc, nc, img, dsum, wy16, val_in, out, C, imm, pads, coeffd, gxd,
           dbg):
    lowpad, a1pad = pads

    def dbg_dump(key, src_ap):
        if key in dbg:
            nd = len(dbg[key].shape)
            nc.sync.dma_start(dbg[key][tuple(slice(None) for _ in range(nd))],
                              src_ap)
    from contextlib import ExitStack

    with ExitStack() as big_ctx:
        wpool = big_ctx.enter_context(tc.tile_pool(name="wpool", bufs=1))
        upool = big_ctx.enter_context(tc.tile_pool(name="upool", bufs=2))
        a8pool = big_ctx.enter_context(tc.tile_pool(name="a8pool", bufs=1))
        w8pool = big_ctx.enter_context(tc.tile_pool(name="w8pool", bufs=1))
        pp = big_ctx.enter_context(tc.tile_pool(name="prep", bufs=1))
        imgp = big_ctx.enter_context(tc.tile_pool(name="imgp", bufs=2))

        # ================= phase A: downsample (local) =====================
        with tc.tile_pool(name="dspool", bufs=2) as dsp:
            import ml_dtypes
            zers = nc.inline_tensor(
                np.zeros(3 * 258 * 258, ml_dtypes.bfloat16), name="zers")
            for pl, cc, ww in ((lowpad, 3, 258), (a1pad, 8, 130)):
                nc.sync.dma_start(bass.AP(pl, 0, [[ww, cc * ww], [1, ww]]),
                                  bass.AP(zers, 0, [[ww, cc * ww], [1, ww]]))
            # lowres[c, i, j] = dsum[c, i, 4j+1] + dsum[c, i, 4j+2]
            # (x0.25 folded into the first conv weights host-side)
            for q2 in range(2):
                for ch in range(3):
                    tp = dsp.tile([128, W], BF16, tag="dsa")
                    nc.sync.dma_start(tp[:],
                                      dsum[ch, 128 * q2:128 * (q2 + 1), :])
                    sb = dsp.tile([128, 256], BF16, tag="lrsb")
                    nc.vector.tensor_tensor(sb[:], tp[:, 1::4], tp[:, 2::4],
                                            OP.add)
                    nc.sync.dma_start(
                        lowpad[ch, 1 + 128 * q2:129 + 128 * q2, 1:257], sb[:])
                    if 'lr' in dbg:
                        nc.sync.dma_start(dbg['lr'][q2 * 3 + ch], sb[:])

        # ---- constant loads: f32 staging in a transient pool ----
        cstg_ctx = ExitStack()
        cstg = cstg_ctx.enter_context(tc.tile_pool(name="cstg", bufs=1))

        def load_const_bf16(name, shape):
            t32 = cstg.tile(list(shape), F32, tag=f"{name}_32")
            nc.sync.dma_start(t32[:], C[name][:])
            tb = wpool.tile(list(shape), BF16, tag=f"{name}_bf")
            nc.vector.tensor_copy(tb[:], t32[:])
            return tb

        def load_const_f32(name, shape):
            t32 = wpool.tile(list(shape), F32, tag=f"{name}_32")
            nc.sync.dma_start(t32[:], C[name][:])
            return t32

        l1w = load_const_bf16("l1w", (9, 24))
        l2w = load_const_bf16("l2w", (24, 48))
        l3w = load_const_bf16("l3w", (48, 96))
        l4w = load_const_bf16("l4w", (96, 192))
        spwT = load_const_bf16("spwT", (64, 64))
        lw1T = load_const_bf16("lw1T", (64, 128))
        lw2T = load_const_bf16("lw2T", (128, 128))
        lw3T = load_const_bf16("lw3T", (128, 64))
        cwT = load_const_bf16("cwT", (64, 4))
        fw1T = load_const_bf16("fw1T", (4, 1024))
        fw2T = load_const_bf16("fw2T", (64, 64))
        gwT = load_const_bf16("gwT", (64, 96))
        sb0 = load_const_f32("sb0", (8, 1))
        sb1 = load_const_f32("sb1", (16, 1))
        sb2 = load_const_f32("sb2", (32, 1))
        sb3 = load_const_f32("sb3", (64, 1))
        spb = load_const_f32("spb", (64, 1))
        lb1 = load_const_f32("lb1", (128, 1))
        lb2 = load_const_f32("lb2", (128, 1))
        lb3 = load_const_f32("lb3", (64, 1))
        cbt = load_const_f32("cb", (4, 1))
        fb1 = load_const_f32("fb1", (64, 1))
        fb2 = load_const_f32("fb2", (64, 1))
        gbt = load_const_f32("gb", (96, 1))
        xib = wpool.tile([16, W], BF16, tag="xib")
        nc.sync.dma_start(xib[:], C["xib"][:])
        # masked y-weight stationary: wytb[p, m, y] = wy16[p%16, y]
        # when (p//16)%4 == m, else 0.
        wytb = wpool.tile([128, 4, HALF], BF16, tag="wytb")
        nc.vector.memset(wytb[:], 0.0)
        for a in range(2):
            for m in range(4):
                nc.sync.dma_start(
                    wytb[64 * a + 16 * m:64 * a + 16 * m + 16, m, :],
                    wy16[:, :])
        # per-z bias constants (-z) for the Act-Abs hat step
        zb = wpool.tile([128, 8], F32, tag="zb")
        for z in range(8):
            nc.vector.memset(zb[:, z:z + 1], -float(z))

        # ========== prep: guide + hat weights for all 4 blocks =============
        # Emitted before the tower so DVE/Act prep overlaps PE tower work.
        # img pool is shared between the guide (f32 reads) and the apply.
        gw_lin = imm["gw_lin"]; gb_lin = imm["gb_lin"]
        U_tiles = []
        img_tiles = []

        for j in range(4):
            r32 = imgp.tile([128, W], F32, tag="r32")
            g32 = imgp.tile([128, W], F32, tag="g32")
            b32 = imgp.tile([128, W], F32, tag="b32")
            nc.sync.dma_start(r32[:], img[0, 128 * j:128 * (j + 1), :])
            nc.sync.dma_start(g32[:], img[1, 128 * j:128 * (j + 1), :])
            nc.sync.dma_start(b32[:], img[2, 128 * j:128 * (j + 1), :])
            img_tiles.append((r32, g32, b32))

            # guide -> cz [128, 1024] f32 (per-channel relus are identities,
            # asserted host-side) — all ops OUT-OF-PLACE.
            t0 = pp.tile([128, W], F32, tag="gt0")
            t1 = pp.tile([128, W], F32, tag="gt1")
            t2 = pp.tile([128, W], F32, tag="gt2")
            cz = pp.tile([128, W], F32, tag="cz")
            nc.vector.tensor_scalar(t0[:], r32[:], float(gw_lin[0]),
                                    float(gb_lin), OP.mult, OP.add)
            nc.vector.scalar_tensor_tensor(
                t1[:], g32[:], float(gw_lin[1]), t0[:], OP.mult, OP.add)
            nc.vector.scalar_tensor_tensor(
                t2[:], b32[:], float(gw_lin[2]), t1[:], OP.mult, OP.add)
            nc.vector.tensor_scalar(cz[:], t2[:], 0.0, 7.0, OP.max, OP.min)
            if j == 0:
                dbg_dump('cz', cz[:])

            # hat weights U[z] = relu(1 - |cz - z|):
            #   a_z = Abs(cz - z)            (Act engine, one per z)
            #   w   = (a min 1) * -1         (DVE TS, 4x mode, whole tile)
            #   U   = w + 1                  (DVE TS, 4x mode, whole tile)
            A8 = a8pool.tile([128, 8, W], BF16, tag="A8")
            for z in range(8):
                nc.scalar.activation(A8[:, z, :], cz[:], AF.Abs,
                                     bias=zb[:, z:z + 1])
            W8 = w8pool.tile([128, 8, W], BF16, tag="W8")
            nc.vector.tensor_scalar(W8[:], A8[:], 1.0, -1.0, OP.min, OP.mult)
            U = upool.tile([128, 8, W], BF16, tag="U")
            nc.vector.tensor_scalar(U[:], W8[:], 1.0, None, OP.add)
            U_tiles.append(U)
            if j == 0:
                dbg_dump('u', U[:, :, :])

        # ================= phase B: conv tower =============================
        with ExitStack() as tower_ctx:
            twp = tower_ctx.enter_context(tc.tile_pool(name="twp", bufs=1))
            ps_big = tower_ctx.enter_context(
                tc.tile_pool(name="ps_big", bufs=1, space="PSUM"))
            ps_med = tower_ctx.enter_context(
                tc.tile_pool(name="ps_med", bufs=1, space="PSUM"))
            ps_small = tower_ctx.enter_context(
                tc.tile_pool(name="ps_small", bufs=2, space="PSUM"))

            # ---- conv1: lowpad -> a1pad, 4 slabs of 32 output rows ----
            with tc.tile_pool(name="c1p", bufs=1) as c1p:
                for s in range(4):
                    im1s = c1p.tile([9, 32, 258], BF16, tag="im1s")
                    for dy in range(3):
                        src = bass.AP(lowpad, (dy + 64 * s) * 258,
                                      [[258 * 258, 3], [2 * 258, 32],
                                       [1, 258]])
                        nc.scalar.dma_start(im1s[dy::3], src)
                    for rr in range(2):
                        r = 2 * s + rr
                        ps = ps_big.tile([8, 2048], F32, tag="psb")
                        for k in range(4):
                            m = rr * 16 + k * 4
                            for dx in range(3):
                                nc.tensor.matmul(
                                    ps[:, k * 512:(k + 1) * 512],
                                    l1w[:, 8 * dx:8 * dx + 8],
                                    im1s[:, m:m + 4, dx:dx + 256:2],
                                    start=(dx == 0), stop=(dx == 2))
                        a1s = c1p.tile([8, 16, 128], BF16, tag="a1s")
                        nc.scalar.activation(a1s[:, :, :], ps[:],
                                             AF.Relu, bias=sb0[:])
                        nc.gpsimd.dma_start(
                            a1pad[:, 1 + 16 * r:17 + 16 * r, 1:129],
                            a1s[:, :, :])

            # ---- conv2: a1pad -> act2 [16,64,64] ----
            with tc.tile_pool(name="c2p", bufs=1) as c2p:
                im2 = c2p.tile([24, 64, 130], BF16, tag="im2")
                for dy in range(3):
                    src = bass.AP(a1pad, dy * 130,
                                  [[130 * 130, 8], [2 * 130, 64], [1, 130]])
                    nc.scalar.dma_start(im2[dy::3], src)
                act2 = c2p.tile([16, 64, 64], BF16, tag="act2")
                for r in range(2):
                    ps = ps_big.tile([16, 2048], F32, tag="psb")
                    for k in range(4):
                        m = r * 32 + k * 8
                        for dx in range(3):
                            nc.tensor.matmul(
                                ps[:, k * 512:(k + 1) * 512],
                                l2w[:, 16 * dx:16 * dx + 16],
                                im2[:, m:m + 8, dx:dx + 128:2],
                                start=(dx == 0), stop=(dx == 2))
                    nc.scalar.activation(act2[:, r * 32:r * 32 + 32, :], ps[:],
                                         AF.Relu, bias=sb1[:])

                # ---- conv3: act2 -> act3 via SBUF-direct im2col scatter ----
                im3 = c2p.tile([48, 32, 66], BF16, tag="im3")
                nc.gpsimd.memset(im3[:], 0.0)
                nc.scalar.dma_start(im3[1::3, 0:32, 1:65], act2[:, 0::2, :])
                nc.scalar.dma_start(im3[2::3, 0:32, 1:65], act2[:, 1::2, :])
                nc.scalar.dma_start(im3[0::3, 1:32, 1:65], act2[:, 1:63:2, :])
                act3 = c2p.tile([32, 32, 32], BF16, tag="act3")
                ps3 = ps_med.tile([32, 1024], F32, tag="psm")
                for k in range(2):
                    for dx in range(3):
                        nc.tensor.matmul(
                            ps3[:, k * 512:(k + 1) * 512],
                            l3w[:, 32 * dx:32 * dx + 32],
                            im3[:, k * 16:k * 16 + 16, dx:dx + 64:2],
                            start=(dx == 0), stop=(dx == 2))
                nc.scalar.activation(act3[:, :, :], ps3[:], AF.Relu,
                                     bias=sb2[:])

                # ---- conv4: act3 -> x4 via SBUF-direct im2col scatter ----
                im4 = c2p.tile([96, 16, 34], BF16, tag="im4")
                nc.gpsimd.memset(im4[:], 0.0)
                nc.scalar.dma_start(im4[1::3, 0:16, 1:33], act3[:, 0::2, :])
                nc.scalar.dma_start(im4[2::3, 0:16, 1:33], act3[:, 1::2, :])
                nc.scalar.dma_start(im4[0::3, 1:16, 1:33], act3[:, 1:31:2, :])
                ps4 = ps_small.tile([64, 256], F32, tag="ps_s")
                for dx in range(3):
                    nc.tensor.matmul(ps4[:], l4w[:, 64 * dx:64 * dx + 64],
                                     im4[:, :, dx:dx + 32:2],
                                     start=(dx == 0), stop=(dx == 2))
                x4 = twp.tile([64, 256], BF16, tag="x4")
                nc.scalar.activation(x4[:], ps4[:], AF.Relu, bias=sb3[:])
                dbg_dump('x4', x4[:])

            # ---- splat = spw @ x4 + spb + val ----
            vt = twp.tile([1, 1], F32, tag="vt")
            nc.scalar.dma_start(vt[:], val_in[:, :])
            vb = twp.tile([64, 1], F32, tag="vb")
            nc.gpsimd.partition_broadcast(vb[:], vt[:])
            spbv = twp.tile([64, 1], F32, tag="spbv")
            nc.gpsimd.tensor_tensor(spbv[:], vb[:], spb[:], OP.add)
            pss = ps_small.tile([64, 256], F32, tag="ps_s")
            nc.tensor.matmul(pss[:], spwT[:], x4[:])
            splat = twp.tile([64, 16, 16], BF16, tag="splat")
            nc.scalar.activation(splat[:, :, :], pss[:], AF.Identity,
                                 bias=spbv[:])
            dbg_dump('splat', splat[:, :, :])

            # ---- local path ----
            psl = ps_small.tile([128, 256], F32, tag="ps_s")
            nc.tensor.matmul(psl[:], lw1T[:], splat[:, :, :])
            loc1 = twp.tile([128, 256], BF16, tag="loc1")
            nc.scalar.activation(loc1[:], psl[:], AF.Relu, bias=lb1[:])
            psl2 = ps_small.tile([128, 256], F32, tag="ps_s")
            nc.tensor.matmul(psl2[:], lw2T[:], loc1[:])
            loc2 = twp.tile([128, 256], BF16, tag="loc2")
            nc.scalar.activation(loc2[:], psl2[:], AF.Relu, bias=lb2[:])
            psl3 = ps_small.tile([64, 256], F32, tag="ps_s")
            nc.tensor.matmul(psl3[:], lw3T[:], loc2[:])
            loc3 = twp.tile([64, 256], BF16, tag="loc3")
            nc.scalar.activation(loc3[:], psl3[:], AF.Relu, bias=lb3[:])

            # ---- condition path ----
            psc = ps_small.tile([4, 64], F32, tag="ps_s")
            nc.tensor.matmul(psc[:], cwT[:], splat[:, 0:16:2, 0:16:2])
            cnd = twp.tile([4, 8, 8], F32, tag="cnd")
            nc.scalar.activation(cnd[:, :, :], psc[:], AF.Relu, bias=cbt[:])
            cp1 = twp.tile([4, 4, 8], F32, tag="cp1")
            nc.gpsimd.tensor_tensor(cp1[:], cnd[:, 0:8:2, :],
                                    cnd[:, 1:8:2, :], OP.add)
            cp2 = twp.tile([4, 4, 4], F32, tag="cp2")
            nc.gpsimd.tensor_tensor(cp2[:], cp1[:, :, 0:8:2],
                                    cp1[:, :, 1:8:2], OP.add)
            cp2b = twp.tile([4, 16], BF16, tag="cp2b")
            nc.gpsimd.tensor_copy(cp2b[:], cp2[:, :, :])
            psf = ps_small.tile([64, 1], F32, tag="ps_s")
            for pos in range(16):
                nc.tensor.matmul(psf[:], fw1T[:, 64 * pos:64 * pos + 64],
                                 cp2b[:, pos:pos + 1],
                                 start=(pos == 0), stop=(pos == 15))
            c1 = twp.tile([64, 1], BF16, tag="c1")
            nc.scalar.activation(c1[:], psf[:], AF.Relu, bias=fb1[:])
            psf2 = ps_small.tile([64, 1], F32, tag="ps_s")
            nc.tensor.matmul(psf2[:], fw2T[:], c1[:])
            c2 = twp.tile([64, 1], F32, tag="c2")
            nc.scalar.activation(c2[:], psf2[:], AF.Relu, bias=fb2[:])

            # ---- fuse + coeff ----
            fused = twp.tile([64, 256], BF16, tag="fused")
            nc.scalar.activation(fused[:], loc3[:], AF.Relu, bias=c2[:])
            psg = ps_small.tile([96, 256], F32, tag="ps_s")
            nc.tensor.matmul(psg[:], gwT[:],
                             fused[:].rearrange("p (gy gx) -> p gx gy",
                                                gy=16, gx=16))
            coeff = twp.tile([96, 256], BF16, tag="coeff")
            nc.scalar.activation(coeff[:], psg[:], AF.Identity, bias=gbt[:])
            nc.scalar.dma_start(coeffd[0:48, :], coeff[0:48, :])
            nc.scalar.dma_start(coeffd[48:96, :], coeff[48:96, :])
            dbg_dump('coeff', coeff[:])

        cstg_ctx.close()

        # G3all [16gx, (96lc', 16gy)] <- coeffd[lc', gy*16+gx], two halves.
        g3 = wpool.tile([16, 1536], BF16, tag="g3")
        for h in range(2):
            src = bass.AP(coeffd, 48 * 256 * h, [[16, 16], [256, 48], [1, 16]])
            nc.scalar.dma_start(g3[:, 768 * h:768 * (h + 1)], src)

        # ================= phase C + D =====================================
        with ExitStack() as main_ctx:
            ps_pair = main_ctx.enter_context(
                tc.tile_pool(name="ps_pair", bufs=2, space="PSUM"))
            mp = main_ctx.enter_context(tc.tile_pool(name="mp", bufs=2))
            stp = main_ctx.enter_context(tc.tile_pool(name="stp", bufs=2))
            mhp = main_ctx.enter_context(tc.tile_pool(name="mhp", bufs=1))
            t4p = main_ctx.enter_context(tc.tile_pool(name="t4p", bufs=1))
            t2p = main_ctx.enter_context(tc.tile_pool(name="t2p", bufs=1))
            affp = main_ctx.enter_context(tc.tile_pool(name="affp", bufs=1))
            gxp = main_ctx.enter_context(tc.tile_pool(name="gxp", bufs=2))

            def emit_phc_pair(t):
                # x-interp of grid rows for coefficients t, t+1 -> DRAM
                ps = ps_pair.tile([128, 2, W], F32, tag="psp")
                for ti in range(2):
                    nc.tensor.matmul(ps[:, ti, 0:512],
                                     g3[:, 128 * (t + ti):128 * (t + ti + 1)],
                                     xib[:, 0:512])
                    nc.tensor.matmul(ps[:, ti, 512:1024],
                                     g3[:, 128 * (t + ti):128 * (t + ti + 1)],
                                     xib[:, 512:1024])
                # drains split Act/DVE to spread the one-time cost
                gx0 = gxp.tile([128, W], BF16, tag="gxs0")
                nc.scalar.activation(gx0[:], ps[:, 0, :], AF.Copy)
                nc.sync.dma_start(gxd[t, :, :], gx0[:])
                gx1 = gxp.tile([128, W], BF16, tag="gxs1")
                nc.vector.tensor_copy(gx1[:], ps[:, 1, :])
                nc.sync.dma_start(gxd[t + 1, :, :], gx1[:])
                if t == 0:
                    dbg_dump('gx', gx0[:])

            for t in range(0, 12, 2):
                emit_phc_pair(t)

            # ================= phase D: main per-block loop ================
            for j in range(4):
                U = U_tiles[j]
                rgb = img_tiles[j]

                aff_tiles = []
                for ci in range(12):
                    gxt = gxp.tile([128, W], BF16, tag="gx")
                    nc.sync.dma_start(gxt[:], gxd[ci, :, :])
                    # y-interp on PE, drains on Act; hat contraction in
                    # z-halves on DVE (2x) with the tail on Pool.
                    Mh = {}
                    for half in range(2):
                        Tst = stp.tile([128, 4, W], BF16, tag="Tst")
                        for zp in range(2):
                            ps = ps_pair.tile([128, 2, W], F32, tag="psp")
                            for zi in range(2):
                                z = 4 * half + 2 * zp + zi
                                hb, m = (z // 4) * 64, z % 4
                                nc.tensor.matmul(
                                    ps[:, zi, 0:512],
                                    wytb[hb:hb + 64, m,
                                         128 * j:128 * (j + 1)],
                                    gxt[hb:hb + 64, 0:512])
                                nc.tensor.matmul(
                                    ps[:, zi, 512:1024],
                                    wytb[hb:hb + 64, m,
                                         128 * j:128 * (j + 1)],
                                    gxt[hb:hb + 64, 512:1024])
                            nc.scalar.activation(
                                Tst[:, 2 * zp:2 * zp + 2, :],
                                ps[:, :, :], AF.Copy)
                        if j == 0 and ci == 0 and half == 0:
                            dbg_dump('tst', Tst[:, :, :])
                        M = mhp.tile([128, 4, W], BF16, tag=f"Mh{half}")
                        nc.vector.tensor_tensor(
                            M[:], Tst[:], U[:, 4 * half:4 * half + 4, :],
                            OP.mult)
                        Mh[half] = M
                    T4 = t4p.tile([128, 4, W], BF16, tag="T4")
                    nc.vector.tensor_tensor(T4[:], Mh[0][:], Mh[1][:], OP.add)
                    T2 = t2p.tile([128, 2, W], BF16, tag="T2")
                    nc.gpsimd.tensor_tensor(T2[:], T4[:, 0:2, :],
                                            T4[:, 2:4, :], OP.add)
                    aff = affp.tile([128, W], BF16, tag=f"aff{ci % 4}")
                    nc.gpsimd.tensor_tensor(aff[:], T2[:, 0, :], T2[:, 1, :],
                                            OP.add)
                    if j == 0 and ci == 0:
                        dbg_dump('aff', aff[:])
                    aff_tiles.append(aff)

                    # apply channel c once its 4 coefficients are ready;
                    # reads the f32 image directly (2x_2p mode), all OOP.
                    if ci % 4 == 3:
                        c = ci // 4
                        a0, a1, a2, a3 = aff_tiles[4 * c:4 * c + 4]
                        m0 = mp.tile([128, W], BF16, tag="apA")
                        m1 = mp.tile([128, W], BF16, tag="apB")
                        nc.vector.tensor_tensor(m0[:], a0[:], rgb[0][:],
                                                OP.mult)
                        nc.vector.tensor_tensor(m1[:], a1[:], rgb[1][:],
                                                OP.mult)
                        s0 = mp.tile([128, W], BF16, tag="apA")
                        nc.vector.tensor_tensor(s0[:], m0[:], m1[:], OP.add)
                        m2 = mp.tile([128, W], BF16, tag="apB")
                        nc.vector.tensor_tensor(m2[:], a2[:], rgb[2][:],
                                                OP.mult)
                        s1 = mp.tile([128, W], BF16, tag="apA")
                        nc.vector.tensor_tensor(s1[:], s0[:], m2[:], OP.add)
                        oc = mp.tile([128, W], F32, tag="oc")
                        nc.vector.tensor_tensor(oc[:], s1[:], a3[:], OP.add)
                        nc.sync.dma_start(out[c, 128 * j:128 * (j + 1), :],
                                          oc[:])


def _host_consts(ip):
    """Build inline-tensor dict + immediates from the input weights."""
    sl = np.asarray(ip['slopes'])[0, :, 0, 0, :]
    sh = np.asarray(ip['shifts'])[:, 0, 0, :]
    assert np.all(sl[:, 1:] == 0.0) and np.all(sl[:, 0] == 1.0), "curve not relu"
    assert np.all(sh[:, 0] == 0.0), "curve not relu"
    prw = np.asarray(ip['prw'])[0]  # [3]
    assert np.all(prw >= 0), "prw must be >= 0 for relu fold"
    ccm_w_h = np.asarray(ip['ccm_w'])
    ccm_b_h = np.asarray(ip['ccm_b'])
    neg_floor = ccm_w_h.clip(max=0.0).sum(axis=1) + ccm_b_h
    assert np.all(neg_floor > -0.01), "guide relu not linearizable"

    t = {}

    def conv_w(w, scale=1.0):
        # w [O, C, 3, 3] -> [3c+dy, 8*dx+o] i.e. [(C*3), (3*O)]
        w = np.asarray(w) * scale
        O, Ci = w.shape[0], w.shape[1]
        m = np.zeros((Ci * 3, 3 * O), np.float32)
        for c in range(Ci):
            for dy in range(3):
                for dx in range(3):
                    m[3 * c + dy, O * dx:O * dx + O] = w[:, c, dy, dx]
        return m

    t['l1w'] = conv_w(ip['sw0'], 0.25)
    t['l2w'] = conv_w(ip['sw1'])
    t['l3w'] = conv_w(ip['sw2'])
    t['l4w'] = conv_w(ip['sw3'])
    t['spwT'] = np.asarray(ip['spw']).T
    t['lw1T'] = np.asarray(ip['lw1']).T
    t['lw2T'] = np.asarray(ip['lw2']).T
    t['lw3T'] = np.asarray(ip['lw3']).T
    t['cwT'] = np.asarray(ip['cw']).T
    fw1 = np.asarray(ip['fw1'])  # [64,64]
    fw1p = np.zeros((4, 16 * 64), np.float32)
    for ch in range(4):
        for pos in range(16):
            fw1p[ch, pos * 64:(pos + 1) * 64] = fw1[:, ch * 16 + pos] * 0.25
    t['fw1T'] = fw1p
    t['fw2T'] = np.asarray(ip['fw2']).T
    # permute g-conv output channels to ci-major (lc' = ci*8+z)
    perm = np.array([z * 12 + ci for ci in range(12) for z in range(8)])
    t['gwT'] = np.asarray(ip['gw']).T[:, perm]
    for n in ('sb0', 'sb1', 'sb2', 'sb3', 'spb', 'lb1', 'lb2', 'lb3',
              'cb', 'fb1', 'fb2'):
        t[n] = np.asarray(ip[n]).reshape(-1, 1)
    t['gb'] = np.asarray(ip['gb'])[perm].reshape(-1, 1)
    t['xi'] = interp_matrix(W, GB)

    prw8_h = 8.0 * prw
    gw_lin = prw8_h @ ccm_w_h                     # [3] weights on (r,g,b)
    gb_lin = float(prw8_h @ ccm_b_h
                   + 8.0 * np.asarray(ip['prb'])[0] - 0.5)
    imm = {
        'gw_lin': gw_lin,
        'gb_lin': gb_lin,
    }
    return {'tensors': t, 'imm': imm}


def _make_in_maps(inputs):
    """Per-core input maps: batch b = k//2, row-half q = k%2."""
    import ml_dtypes
    ip = {k: np.asarray(v) for k, v in inputs.items()}
    wy_full = interp_matrix(H, GB)  # [16, 1024]
    wy16 = [np.ascontiguousarray(
        wy_full[:, HALF * q:HALF * (q + 1)]).astype(ml_dtypes.bfloat16)
        for q in range(2)]
    img = ip['image']
    # full-image row-pair sums for the bilinear 4x downsample
    dsum = (img[:, :, 1::4, :].astype(np.float32)
            + img[:, :, 2::4, :]).astype(ml_dtypes.bfloat16)  # [B,3,256,W]
    in_maps = []
    for k in range(N_CORES):
        b, q = k // 2, k % 2
        in_maps.append({
            "img": img[b, :, HALF * q:HALF * (q + 1), :].copy(),
            "dsum": np.ascontiguousarray(dsum[b]),
            "wy16": wy16[q],
            "val": ip['val'][b].reshape(1, 1).copy(),
        })
    return in_maps


_CACHE = {}


def kernel(**inputs):
    ip = {k: np.asarray(v) for k, v in inputs.items()}
    import hashlib
    h = hashlib.sha1()
    for k in sorted(ip):
        if k in ('image', 'val'):
            continue
        h.update(k.encode())
        h.update(np.ascontiguousarray(ip[k]).tobytes())
    key = h.hexdigest()
    if key in _CACHE:
        nc = _CACHE[key]
    else:
        consts = _host_consts(ip)
        nc = _build_nc(consts)
        _CACHE[key] = nc

    in_maps = _make_in_maps(ip)
    res = run_bass_kernel_spmd(nc, in_maps, core_ids=list(range(N_CORES)))
    full = np.zeros((B, NIN, H, W), np.float32)
    for k in range(N_CORES):
        b, q = k // 2, k % 2
        full[b, :, HALF * q:HALF * (q + 1), :] = res.results[k]["out"]
    return full


if __name__ == "__main__":
    import jax
    jax.config.update('jax_platforms', 'cpu')
    sys.path.insert(0, '/root/problem')
    import reference as R
    inputs = R.setup_inputs()
    outp = kernel(**{k: np.asarray(v) for k, v in inputs.items()})
    print("kernel out", outp.shape)
